# revision 46
# baseline (speedup 1.0000x reference)
"""Multi-head attention forward on 8 Trainium2 NeuronCores.

Reference computation (B=2, N=2048, C=1024, H=16, Dh=64):
    qkv = x @ qkv_w.T + qkv_b            -> q, k, v per head
    attn = softmax(q @ k.T / sqrt(Dh))
    out  = (attn @ v) reassembled, then out @ proj_w.T + proj_b

Sharding: 8 cores = 2 batches x 4 head groups (tensor parallel on heads,
data parallel on batch).  Each core computes q/k/v for its 4 heads over
its batch's 2048 tokens, attention for those heads, and a partial
projection with its head-group's rows of proj_w.  The host sums the 4
partial projections per batch (bf16 partials, f32 accumulate) and adds
the (host-folded) proj + v biases.

Schedule: everything runs in the S^T orientation (S^T[j,i] = sum_d
kT[d,j] qT[d,i]) so softmax reductions over keys happen via matmul -- a
ones column in v-hat yields the denominator as row 64 of the AV PSUM
tile.  The kernel sits on the PE/ACT ridge (~155us PE floor, ~147us ACT
floor), so the schedule is chunk-major and PAIR-INTERLEAVED: head pairs
alternate per query chunk so both pairs of a chunk finish close together
and the projection + y DMA spread through the whole kernel instead of
piling up in a drain tail.  exp'd score tiles (es) buffer in SBUF,
decoupling the S+exp stream from the AV stream.  All matmul operands
are bf16.  Normalization evicts the ones-row + raw ao from PSUM
immediately (freeing the AV accumulators), takes the reciprocal on DVE,
broadcasts it across partitions on gpsimd, and rescales into aoT -- no
PE broadcast matmuls.  y is written bf16 (host sums partials in f32),
halving the output DMA.  Softmax max-subtraction is skipped
(S ~ N(0,1)).  The k bias is dropped (softmax-invariant); v/proj biases
are folded on the host.
"""

import sys

if "/opt/trn_rl_repo" not in sys.path:
    sys.path.insert(0, "/opt/trn_rl_repo")

from contextlib import ExitStack

import ml_dtypes
import numpy as np

from concourse import bacc, mybir, tile
from concourse.bass_utils import run_bass_kernel_spmd

F32 = mybir.dt.float32
BF16 = mybir.dt.bfloat16
AF = mybir.ActivationFunctionType

B, N, C, H, DH = 2, 2048, 1024, 16, 64
NCORES = 8
HG = 4              # head groups (cores per batch)
HPG = H // HG       # 4 heads per core
DG = HPG * DH       # 256 projected dims per core
CT = C // 128       # 8 contraction tiles
JT = N // 128       # 16 key tiles
IC = N // 512       # 4 query chunks
SCALE = DH ** -0.5

_CACHE = {}
LAST_RESULTS = None


def _build():
    nc = bacc.Bacc("TRN2", target_bir_lowering=False, debug=False,
                   num_devices=NCORES)

    # x quarter-major on BOTH sides: [128, quarter, ct, 512] so each
    # 512-token quarter is one DMA with contiguous 8KB per-partition lines
    xT = nc.dram_tensor("xT", [128, 4, CT, 512], BF16, kind="ExternalInput").ap()
    wq0 = nc.dram_tensor("wq0", [128, CT, 128], BF16, kind="ExternalInput").ap()
    wq1 = nc.dram_tensor("wq1", [128, CT, 128], BF16, kind="ExternalInput").ap()
    wk0 = nc.dram_tensor("wk0", [128, CT, 128], BF16, kind="ExternalInput").ap()
    wk1 = nc.dram_tensor("wk1", [128, CT, 128], BF16, kind="ExternalInput").ap()
    wv = nc.dram_tensor("wv", [128, CT, DG], BF16, kind="ExternalInput").ap()
    wp = nc.dram_tensor("wp", [128, DG // 128, C], BF16, kind="ExternalInput").ap()
    qbT = nc.dram_tensor("qbT", [128, 2], F32, kind="ExternalInput").ap()
    y = nc.dram_tensor("y", [N, C], BF16, kind="ExternalOutput").ap()

    with tile.TileContext(nc) as tc, ExitStack() as ctx:
        per = ctx.enter_context(tc.tile_pool(name="per", bufs=1))
        xT_s = per.tile([128, 4, CT, 512], BF16, tag="xT")
        qT_s = per.tile([128, 2, N], BF16, tag="qT")
        kT_s = per.tile([128, 2, N], BF16, tag="kT")
        vh_s = per.tile([128, JT, HPG, DH + 1], BF16, tag="vh")
        aoT_s = per.tile([128, 2, N], BF16, tag="aoT")
        wq0_t = per.tile([128, CT, 128], BF16, tag="wq0")
        wq1_t = per.tile([128, CT, 128], BF16, tag="wq1")
        wk0_t = per.tile([128, CT, 128], BF16, tag="wk0")
        wk1_t = per.tile([128, CT, 128], BF16, tag="wk1")
        wv_t = per.tile([128, CT, DG], BF16, tag="wv")
        wp_t = per.tile([128, DG // 128, C], BF16, tag="wp")
        qbT_s = per.tile([128, 2], F32, tag="qbT")
        ones_s = per.tile([1, 512], BF16, tag="ones")
        warm = per.tile([1, 16], F32, tag="warm")

        # ---- DMA emission.  Queues: sync + gpsimd (cheap trigger) +
        # vector carry inputs; the scalar (ACT) queue stays free for exp.
        def xdma(q, quarter, ct0, ct1):
            q.dma_start(xT_s[:, quarter, ct0:ct1, :],
                        xT[:, quarter, ct0:ct1, :])

        # Per-queue bandwidth is ~130-160 GB/s, so the critical first-chunk
        # set (wk0 + wq0 + x quarter 0) is split across all three DMA-capable
        # queues, and the x quarters stream in consumption order.  ones is
        # memset on-device (no DMA) so the warm matmuls start immediately.
        nc.gpsimd.memset(ones_s[:], 1.0)
        nc.sync.dma_start(wk0_t[:], wk0)
        nc.gpsimd.dma_start(wq0_t[:], wq0)
        xdma(nc.scalar, 0, 0, 3)
        xdma(nc.sync, 0, 3, 6)
        xdma(nc.gpsimd, 0, 6, 8)
        nc.gpsimd.dma_start(qbT_s[:], qbT)
        nc.scalar.dma_start(wk1_t[:], wk1)
        xdma(nc.scalar, 1, 0, 4)
        xdma(nc.gpsimd, 1, 4, 8)
        nc.gpsimd.dma_start(wq1_t[:], wq1)
        xdma(nc.scalar, 2, 0, 4)
        xdma(nc.sync, 2, 4, 8)
        nc.sync.dma_start(wv_t[:], wv)
        xdma(nc.gpsimd, 3, 0, 4)
        xdma(nc.scalar, 3, 4, 8)
        nc.gpsimd.dma_start(wp_t[:], wp)

        with tc.tile_pool(name="es", bufs=24) as esp, \
             tc.tile_pool(name="sm", bufs=4) as sm2, \
             tc.tile_pool(name="yp", bufs=3) as yp, \
             tc.tile_pool(name="psA", bufs=2, space="PSUM") as psA, \
             tc.tile_pool(name="psS", bufs=2, space="PSUM") as psS, \
             tc.tile_pool(name="psB", bufs=2, space="PSUM") as psB:

            # warm the exp table while the bulk DMAs run, and spin the
            # PE clock up to full rate (dense dummy matmuls overlap the
            # first x-chunk DMAs, so production starts warm)
            nc.scalar.activation(warm[:], ones_s[:, 0:16], AF.Exp)
            jk0 = psA.tile([128, 512], F32, tag="mm", name="jk0")
            for _ in range(6):
                nc.tensor.matmul(jk0[:], ones_s[:, 0:128], ones_s[:],
                                 start=True, stop=True)
            nc.gpsimd.memset(vh_s[:, :, :, DH], 1.0)

            def qk_chunk(w_t, dst, dt, nck, bias=False):
                ps = psA.tile([128, 512], F32, tag="mm")
                for ct in range(CT):
                    nc.tensor.matmul(
                        ps[:], w_t[:, ct, :],
                        xT_s[:, nck, ct, :],
                        start=(ct == 0), stop=(ct == CT - 1))
                out = dst[:, dt, nck * 512:(nck + 1) * 512]
                if bias:
                    nc.vector.tensor_scalar_add(out, ps[:],
                                                qbT_s[:, dt:dt + 1])
                else:
                    nc.vector.tensor_copy(out, ps[:])

            def vhat(jt):
                ps = psA.tile([128, 512], F32, tag="mm")
                q, r = divmod(jt, 4)
                for ct in range(CT):
                    nc.tensor.matmul(ps[:, 0:DG],
                                     xT_s[:, q, ct, r * 128:(r + 1) * 128],
                                     wv_t[:, ct, :],
                                     start=(ct == 0), stop=(ct == CT - 1))
                for h in range(HPG):
                    nc.vector.tensor_copy(vh_s[:, jt, h, 0:DH],
                                          ps[:, h * DH:(h + 1) * DH])

            es_store = {}
            av_tiles = {}

            def se_pair(p, ic, jc):
                # S^T row-group pair + exp for one key tile of chunk ic
                i0 = ic * 512
                st = psS.tile([128, 1024], F32, tag="st")
                nc.tensor.matmul(st[:, 0:512],
                                 kT_s[0:64, p, jc * 128:(jc + 1) * 128],
                                 qT_s[0:64, p, i0:i0 + 512],
                                 start=True, stop=True)
                nc.tensor.matmul(st[:, 512:1024],
                                 kT_s[64:128, p, jc * 128:(jc + 1) * 128],
                                 qT_s[64:128, p, i0:i0 + 512],
                                 start=True, stop=True)
                es = esp.tile([128, 1024], BF16, tag="es",
                              name=f"es{p}_{ic}_{jc}")
                es_store[(p, ic, jc)] = es
                nc.scalar.activation(es[:], st[:], AF.Exp, scale=SCALE)

            def av(p, ic, jc0, jc1, hs=(0, 1)):
                if (p, ic) not in av_tiles:
                    av_tiles[(p, ic)] = [
                        psB.tile([DH + 1, 512], F32, tag="outT",
                                 name=f"o{p}_{ic}{s}") for s in "ab"]
                outs = av_tiles[(p, ic)]
                for jc in range(jc0, jc1):
                    es = (es_store.pop((p, ic, jc)) if 1 in hs
                          else es_store[(p, ic, jc)])
                    for h in hs:
                        nc.tensor.matmul(
                            outs[h][:], vh_s[:, jc, 2 * p + h, :],
                            es[:, h * 512:(h + 1) * 512],
                            start=(jc == 0), stop=(jc == JT - 1))

            def norm(p, ic, hs=(0, 1)):
                # evict den + raw ao immediately (frees the psB tiles for
                # the next column's AV), then reciprocal on DVE, broadcast
                # on gpsimd, and normalize into aoT.  No PE work.
                i0 = ic * 512
                outs = av_tiles[(p, ic)]
                if 1 in hs:
                    av_tiles.pop((p, ic))
                raws, recs = [], []
                for hi in hs:
                    outT = outs[hi]
                    den = sm2.tile([1, 512], F32, tag="den")
                    nc.vector.tensor_copy(den[:], outT[64:65, :])
                    raw = sm2.tile([64, 512], F32, tag="raw")
                    nc.vector.tensor_copy(raw[:], outT[0:64, :])
                    rec = sm2.tile([1, 512], F32, tag="rec")
                    nc.vector.reciprocal_approx_fast(rec[:], den[:])
                    raws.append(raw)
                    recs.append(rec)
                bcs = []
                for k in range(len(hs)):
                    bc = sm2.tile([64, 512], F32, tag="bc")
                    nc.gpsimd.partition_broadcast(bc[:], recs[k][:])
                    bcs.append(bc)
                for k, hi in enumerate(hs):
                    ao = aoT_s[hi * 64:hi * 64 + 64, p, i0:i0 + 512]
                    nc.vector.tensor_mul(ao, raws[k][:], bcs[k][:])

            # y DMA queues: sync/gpsimd alternate; the last column's tiles
            # fan out over three queues (ACT is idle by then)
            def yqueue(it):
                if it < 12:
                    return nc.sync if it % 2 == 0 else nc.gpsimd
                return {12: nc.sync, 13: nc.gpsimd,
                        14: nc.scalar, 15: nc.sync}[it]

            yt_cur = {}

            def proj_half(it, ec):
                # half a 128-row tile of y: 2 accumulating matmuls, cast,
                # and (on the second half) the row-tile DMA.  One psA tile
                # per half so consecutive halves pipeline on the 2-deep
                # 'mm' ring.
                ps = psA.tile([128, 512], F32, tag="mm", name=f"pj{it}_{ec}")
                for dt in range(DG // 128):
                    nc.tensor.matmul(
                        ps[:],
                        aoT_s[:, dt, it * 128:(it + 1) * 128],
                        wp_t[:, dt, ec * 512:(ec + 1) * 512],
                        start=(dt == 0), stop=(dt == DG // 128 - 1))
                if ec == 0:
                    yt_cur[it] = yp.tile([128, C], BF16, tag="y",
                                         name=f"yt{it}")
                yt = yt_cur[it]
                nc.vector.tensor_copy(yt[:, ec * 512:(ec + 1) * 512], ps[:])
                if ec == 1:
                    yt_cur.pop(it)
                    yqueue(it).dma_start(y[it * 128:(it + 1) * 128, :], yt[:])

            # ---- emission schedule: chunk-major, pair-interleaved -------
            # column order (0,0),(1,0),(0,1),(1,1),... ; se leads av by
            # TRAIL steps; q/k production and vhats are emitted just in
            # time; after both pairs of a chunk norm, its 4 proj tiles
            # drip out one per step.  The first 8 se blocks are permuted
            # so each block's x quarters (kT chunk b, qT chunk ic) have
            # landed by the time the PE reaches it -- the exp stream then
            # runs stall-free from ~13us.
            COLS = [(p, ic) for ic in range(IC) for p in range(2)]
            av_list = [(p, ic, b) for (p, ic) in COLS for b in range(4)]
            se_order = [(0, 0, 0), (0, 0, 1), (1, 0, 0), (0, 0, 2),
                        (1, 0, 1), (0, 0, 3), (1, 0, 2), (1, 0, 3)] + \
                       [(p, ic, b) for (p, ic) in COLS[2:] for b in range(4)]
            TRAIL = 3
            se_step = {}

            k_done, q_done, vh_done = set(), set(), set()
            normed = set()
            proj_pending = []

            WK = {0: wk0_t, 1: wk1_t}
            WQ = {0: wq0_t, 1: wq1_t}

            def need_se(p, ic, b):
                if (p, b) not in k_done:
                    k_done.add((p, b))
                    qk_chunk(WK[p], kT_s, p, b)
                if (p, ic) not in q_done:
                    q_done.add((p, ic))
                    qk_chunk(WQ[p], qT_s, p, ic, bias=True)

            def need_vh(blk):
                if blk not in vh_done:
                    vh_done.add(blk)
                    for jt in range(4 * blk, 4 * blk + 4):
                        vhat(jt)

            def do_av(idx, half):
                p, ic, b = av_list[idx]
                need_vh(b)
                av(p, ic, 4 * b + 2 * half, 4 * b + 2 * half + 2)
                if b == 3 and half == 1:
                    norm(p, ic)
                    normed.add((p, ic))
                    if (1 - p, ic) in normed:
                        proj_pending.extend(
                            (it, ec) for it in range(4 * ic, 4 * ic + 4)
                            for ec in range(2))

            L = len(av_list)
            ai = 0
            need_se(*se_order[0])
            for i, sblk in enumerate(se_order):
                p, ic, b = sblk
                for jc in range(4 * b, 4 * b + 4):
                    se_pair(p, ic, jc)
                se_step[sblk] = i

                def av_ready():
                    return (ai < L - 2 and av_list[ai] in se_step
                            and se_step[av_list[ai]] <= i - TRAIL)

                pops = 0
                prefetched = False
                while av_ready() and pops < 2:
                    do_av(ai, 0)
                    # production for LATER steps sits between the av
                    # halves so ACT has stream to chew meanwhile
                    if not prefetched and i + 1 < len(se_order):
                        need_se(*se_order[i + 1])
                        prefetched = True
                    do_av(ai, 1)
                    ai += 1
                    pops += 1
                if not prefetched and i + 1 < len(se_order):
                    need_se(*se_order[i + 1])
                if ai < L:
                    need_vh(av_list[ai][2])
                if proj_pending:
                    proj_half(*proj_pending.pop(0))
            while ai < L - 2:
                do_av(ai, 0)
                do_av(ai, 1)
                ai += 1
                if proj_pending:
                    proj_half(*proj_pending.pop(0))
            # epilogue: the last column's remaining av blocks run per-head
            # so h0's norm chain (DVE/gpsimd) overlaps h1's AV matmuls
            for hs in ((0,), (1,)):
                for j in range(L - 2, L):
                    p_, ic_, b_ = av_list[j]
                    av(p_, ic_, 4 * b_, 4 * b_ + 4, hs=hs)
                norm(p_, ic_, hs=hs)
            normed.add((p_, ic_))
            proj_pending.extend(
                (it, ec) for it in range(4 * ic_, 4 * ic_ + 4)
                for ec in range(2))
            while proj_pending:
                proj_half(*proj_pending.pop(0))

    nc.compile()
    return nc


def _get_nc():
    if "nc" not in _CACHE:
        _CACHE["nc"] = _build()
    return _CACHE["nc"]


def kernel(x, qkv_w, qkv_b, proj_w, proj_b):
    global LAST_RESULTS
    x = np.asarray(x, dtype=np.float32)
    qkv_w = np.asarray(qkv_w, dtype=np.float32)
    qkv_b = np.asarray(qkv_b, dtype=np.float32)
    proj_w = np.asarray(proj_w, dtype=np.float32)
    proj_b = np.asarray(proj_b, dtype=np.float32)

    nc = _get_nc()
    bf16 = ml_dtypes.bfloat16

    wqT_f = qkv_w[0:C].T                # [C, C]
    wkT_f = qkv_w[C:2 * C].T
    wvT_f = qkv_w[2 * C:3 * C].T
    wpT_f = proj_w.T                    # [C, C]

    def tile128(a):
        # [C, W] -> [128, CT, W] with partition = c % 128, ct = c // 128
        w = a.shape[1]
        return np.ascontiguousarray(
            a.reshape(CT, 128, w).transpose(1, 0, 2))

    in_maps = []
    for c in range(NCORES):
        b, g = divmod(c, HG)
        ds = g * DG
        wq_g = tile128(wqT_f[:, ds:ds + DG]).astype(bf16)  # [128, CT, 256]
        wk_g = tile128(wkT_f[:, ds:ds + DG]).astype(bf16)
        wp_g = np.ascontiguousarray(
            wpT_f[ds:ds + DG].reshape(2, 128, C).transpose(1, 0, 2)).astype(bf16)
        # qbT: per-partition q bias, column dt = head pair
        qbT = np.ascontiguousarray(
            qkv_b[ds:ds + DG].reshape(2, 128).T, dtype=np.float32)
        # xT quarter-major: [128, 4, CT, 512]; partition = c % 128
        xq = x[b].T.reshape(CT, 128, 4, 512).transpose(1, 2, 0, 3)
        in_maps.append({
            "xT": np.ascontiguousarray(xq).astype(bf16),
            "wq0": np.ascontiguousarray(wq_g[:, :, 0:128]),
            "wq1": np.ascontiguousarray(wq_g[:, :, 128:256]),
            "wk0": np.ascontiguousarray(wk_g[:, :, 0:128]),
            "wk1": np.ascontiguousarray(wk_g[:, :, 128:256]),
            "wv": tile128(wvT_f[:, ds:ds + DG]).astype(bf16),
            "wp": wp_g,
            "qbT": qbT,
        })

    LAST_RESULTS = run_bass_kernel_spmd(nc, in_maps, list(range(NCORES)))
    # host unshard: sum the 4 partial projections per batch (f32 accumulate
    # of bf16 partials) and add the folded bias (proj_b + v_bias @ proj_w.T
    # -- exact, since sum(attn)=1)
    out_bias = proj_b + qkv_b[2 * C:3 * C] @ proj_w.T
    out = np.empty((B, N, C), np.float32)
    for b in range(B):
        acc = LAST_RESULTS.results[b * HG]["y"].astype(np.float32)
        for g in range(1, HG):
            acc = acc + LAST_RESULTS.results[b * HG + g]["y"].astype(np.float32)
        out[b] = acc + out_bias
    return out


# revision 48
# speedup vs baseline: 1.0145x; 1.0145x over previous
"""Multi-head attention forward on 8 Trainium2 NeuronCores.

Reference computation (B=2, N=2048, C=1024, H=16, Dh=64):
    qkv = x @ qkv_w.T + qkv_b            -> q, k, v per head
    attn = softmax(q @ k.T / sqrt(Dh))
    out  = (attn @ v) reassembled, then out @ proj_w.T + proj_b

Sharding: 8 cores = 2 batches x 4 head groups (tensor parallel on heads,
data parallel on batch).  Each core computes q/k/v for its 4 heads over
its batch's 2048 tokens, attention for those heads, and a partial
projection with its head-group's rows of proj_w.  The host sums the 4
partial projections per batch (bf16 partials, f32 accumulate) and adds
the (host-folded) proj + v biases.

Schedule: everything runs in the S^T orientation (S^T[j,i] = sum_d
kT[d,j] qT[d,i]) so softmax reductions over keys happen via matmul -- a
ones column in v-hat yields the denominator as row 64 of the AV PSUM
tile.  The kernel sits on the PE/ACT ridge (~155us PE floor, ~147us ACT
floor), so the schedule is chunk-major and PAIR-INTERLEAVED: head pairs
alternate per query chunk so both pairs of a chunk finish close together
and the projection + y DMA spread through the whole kernel instead of
piling up in a drain tail.  exp'd score tiles (es) buffer in SBUF,
decoupling the S+exp stream from the AV stream.  All matmul operands
are bf16.  Normalization evicts the ones-row + raw ao from PSUM
immediately (freeing the AV accumulators), takes the reciprocal on DVE,
broadcasts it across partitions on gpsimd, and rescales into aoT -- no
PE broadcast matmuls.  y is written bf16 (host sums partials in f32),
halving the output DMA.  Softmax max-subtraction is skipped
(S ~ N(0,1)).  The k bias is dropped (softmax-invariant); v/proj biases
are folded on the host.
"""

import sys

if "/opt/trn_rl_repo" not in sys.path:
    sys.path.insert(0, "/opt/trn_rl_repo")

from contextlib import ExitStack

import ml_dtypes
import numpy as np

from concourse import bacc, mybir, tile
from concourse.bass_utils import run_bass_kernel_spmd

F32 = mybir.dt.float32
BF16 = mybir.dt.bfloat16
AF = mybir.ActivationFunctionType

B, N, C, H, DH = 2, 2048, 1024, 16, 64
NCORES = 8
HG = 4              # head groups (cores per batch)
HPG = H // HG       # 4 heads per core
DG = HPG * DH       # 256 projected dims per core
CT = C // 128       # 8 contraction tiles
JT = N // 128       # 16 key tiles
IC = N // 512       # 4 query chunks
SCALE = DH ** -0.5

_CACHE = {}
LAST_RESULTS = None


def _build():
    nc = bacc.Bacc("TRN2", target_bir_lowering=False, debug=False,
                   num_devices=NCORES)

    # x quarter-major on BOTH sides: [128, quarter, ct, 512] so each
    # 512-token quarter is one DMA with contiguous 8KB per-partition lines
    xT = nc.dram_tensor("xT", [128, 4, CT, 512], BF16, kind="ExternalInput").ap()
    wq0 = nc.dram_tensor("wq0", [128, CT, 128], BF16, kind="ExternalInput").ap()
    wq1 = nc.dram_tensor("wq1", [128, CT, 128], BF16, kind="ExternalInput").ap()
    wk0 = nc.dram_tensor("wk0", [128, CT, 128], BF16, kind="ExternalInput").ap()
    wk1 = nc.dram_tensor("wk1", [128, CT, 128], BF16, kind="ExternalInput").ap()
    wv = nc.dram_tensor("wv", [128, CT, DG], BF16, kind="ExternalInput").ap()
    wp = nc.dram_tensor("wp", [128, DG // 128, C], BF16, kind="ExternalInput").ap()
    qbT = nc.dram_tensor("qbT", [128, 2], F32, kind="ExternalInput").ap()
    y = nc.dram_tensor("y", [N, C], BF16, kind="ExternalOutput").ap()

    with tile.TileContext(nc) as tc, ExitStack() as ctx:
        per = ctx.enter_context(tc.tile_pool(name="per", bufs=1))
        xT_s = per.tile([128, 4, CT, 512], BF16, tag="xT")
        qT_s = per.tile([128, 2, N], BF16, tag="qT")
        kT_s = per.tile([128, 2, N], BF16, tag="kT")
        vh_s = per.tile([128, JT, HPG, DH + 1], BF16, tag="vh")
        aoT_s = per.tile([128, 2, N], BF16, tag="aoT")
        wq0_t = per.tile([128, CT, 128], BF16, tag="wq0")
        wq1_t = per.tile([128, CT, 128], BF16, tag="wq1")
        wk0_t = per.tile([128, CT, 128], BF16, tag="wk0")
        wk1_t = per.tile([128, CT, 128], BF16, tag="wk1")
        wv_t = per.tile([128, CT, DG], BF16, tag="wv")
        wp_t = per.tile([128, DG // 128, C], BF16, tag="wp")
        qbT_s = per.tile([128, 2], F32, tag="qbT")
        ones_s = per.tile([1, 512], BF16, tag="ones")
        warm = per.tile([1, 16], F32, tag="warm")

        # ---- DMA emission.  Queues: sync + gpsimd (cheap trigger) +
        # vector carry inputs; the scalar (ACT) queue stays free for exp.
        def xdma(q, quarter, ct0, ct1):
            q.dma_start(xT_s[:, quarter, ct0:ct1, :],
                        xT[:, quarter, ct0:ct1, :])

        # Per-queue bandwidth is ~130-160 GB/s, so the critical first-chunk
        # set (wk0 + wq0 + x quarter 0) is split across all three DMA-capable
        # queues, and the x quarters stream in consumption order.  ones is
        # memset on-device (no DMA) so the warm matmuls start immediately.
        nc.gpsimd.memset(ones_s[:], 1.0)
        nc.sync.dma_start(wk0_t[:], wk0)
        nc.gpsimd.dma_start(wq0_t[:], wq0)
        xdma(nc.scalar, 0, 0, 3)
        xdma(nc.sync, 0, 3, 6)
        xdma(nc.gpsimd, 0, 6, 8)
        nc.gpsimd.dma_start(qbT_s[:], qbT)
        nc.scalar.dma_start(wk1_t[:], wk1)
        xdma(nc.scalar, 1, 0, 4)
        xdma(nc.gpsimd, 1, 4, 8)
        nc.gpsimd.dma_start(wq1_t[:], wq1)
        xdma(nc.scalar, 2, 0, 4)
        xdma(nc.sync, 2, 4, 8)
        nc.sync.dma_start(wv_t[:], wv)
        xdma(nc.gpsimd, 3, 0, 4)
        xdma(nc.scalar, 3, 4, 8)
        nc.gpsimd.dma_start(wp_t[:], wp)

        with tc.tile_pool(name="es", bufs=24) as esp, \
             tc.tile_pool(name="sm", bufs=4) as sm2, \
             tc.tile_pool(name="yp", bufs=3) as yp, \
             tc.tile_pool(name="psA", bufs=2, space="PSUM") as psA, \
             tc.tile_pool(name="psS", bufs=2, space="PSUM") as psS, \
             tc.tile_pool(name="psB", bufs=2, space="PSUM") as psB:

            # warm the exp table while the bulk DMAs run, and spin the
            # PE clock up to full rate (dense dummy matmuls overlap the
            # first x-chunk DMAs, so production starts warm)
            nc.scalar.activation(warm[:], ones_s[:, 0:16], AF.Exp)
            jk0 = psA.tile([128, 512], F32, tag="mm", name="jk0")
            for _ in range(6):
                nc.tensor.matmul(jk0[:], ones_s[:, 0:128], ones_s[:],
                                 start=True, stop=True)
            nc.gpsimd.memset(vh_s[:, :, :, DH], 1.0)

            def qk_chunk(w_t, dst, dt, nck, bias=False):
                ps = psA.tile([128, 512], F32, tag="mm")
                for ct in range(CT):
                    nc.tensor.matmul(
                        ps[:], w_t[:, ct, :],
                        xT_s[:, nck, ct, :],
                        start=(ct == 0), stop=(ct == CT - 1))
                out = dst[:, dt, nck * 512:(nck + 1) * 512]
                if bias:
                    nc.vector.tensor_scalar_add(out, ps[:],
                                                qbT_s[:, dt:dt + 1])
                else:
                    nc.vector.tensor_copy(out, ps[:])

            def vhat(jt):
                ps = psA.tile([128, 512], F32, tag="mm")
                q, r = divmod(jt, 4)
                for ct in range(CT):
                    nc.tensor.matmul(ps[:, 0:DG],
                                     xT_s[:, q, ct, r * 128:(r + 1) * 128],
                                     wv_t[:, ct, :],
                                     start=(ct == 0), stop=(ct == CT - 1))
                for h in range(HPG):
                    nc.vector.tensor_copy(vh_s[:, jt, h, 0:DH],
                                          ps[:, h * DH:(h + 1) * DH])

            es_store = {}
            av_tiles = {}

            def se_pair(p, ic, jc):
                # S^T row-group pair + exp for one key tile of chunk ic
                i0 = ic * 512
                st = psS.tile([128, 1024], F32, tag="st")
                nc.tensor.matmul(st[:, 0:512],
                                 kT_s[0:64, p, jc * 128:(jc + 1) * 128],
                                 qT_s[0:64, p, i0:i0 + 512],
                                 start=True, stop=True)
                nc.tensor.matmul(st[:, 512:1024],
                                 kT_s[64:128, p, jc * 128:(jc + 1) * 128],
                                 qT_s[64:128, p, i0:i0 + 512],
                                 start=True, stop=True)
                es = esp.tile([128, 1024], BF16, tag="es",
                              name=f"es{p}_{ic}_{jc}")
                es_store[(p, ic, jc)] = es
                nc.scalar.activation(es[:], st[:], AF.Exp, scale=SCALE)

            def av(p, ic, jc0, jc1, hs=(0, 1)):
                if (p, ic) not in av_tiles:
                    av_tiles[(p, ic)] = [
                        psB.tile([DH + 1, 512], F32, tag="outT",
                                 name=f"o{p}_{ic}{s}") for s in "ab"]
                outs = av_tiles[(p, ic)]
                for jc in range(jc0, jc1):
                    es = (es_store.pop((p, ic, jc)) if 1 in hs
                          else es_store[(p, ic, jc)])
                    for h in hs:
                        nc.tensor.matmul(
                            outs[h][:], vh_s[:, jc, 2 * p + h, :],
                            es[:, h * 512:(h + 1) * 512],
                            start=(jc == 0), stop=(jc == JT - 1))

            def norm(p, ic, hs=(0, 1)):
                # evict den + raw ao immediately (frees the psB tiles for
                # the next column's AV), then reciprocal on DVE, broadcast
                # on gpsimd, and normalize into aoT.  No PE work.
                i0 = ic * 512
                outs = av_tiles[(p, ic)]
                if 1 in hs:
                    av_tiles.pop((p, ic))
                raws, recs = [], []
                for hi in hs:
                    outT = outs[hi]
                    den = sm2.tile([1, 512], F32, tag="den")
                    nc.vector.tensor_copy(den[:], outT[64:65, :])
                    raw = sm2.tile([64, 512], F32, tag="raw")
                    nc.vector.tensor_copy(raw[:], outT[0:64, :])
                    rec = sm2.tile([1, 512], F32, tag="rec")
                    nc.vector.reciprocal_approx_fast(rec[:], den[:])
                    raws.append(raw)
                    recs.append(rec)
                bcs = []
                for k in range(len(hs)):
                    bc = sm2.tile([64, 512], F32, tag="bc")
                    nc.gpsimd.partition_broadcast(bc[:], recs[k][:])
                    bcs.append(bc)
                for k, hi in enumerate(hs):
                    ao = aoT_s[hi * 64:hi * 64 + 64, p, i0:i0 + 512]
                    nc.vector.tensor_mul(ao, raws[k][:], bcs[k][:])

            # y DMA queues: sync/gpsimd alternate; the last column's tiles
            # fan out over three queues (ACT is idle by then)
            def yqueue(it):
                if it < 12:
                    return nc.sync if it % 2 == 0 else nc.gpsimd
                return {12: nc.sync, 13: nc.gpsimd,
                        14: nc.scalar, 15: nc.sync}[it]

            yt_cur = {}

            def proj_half(it, ec):
                # half a 128-row tile of y: 2 accumulating matmuls, cast,
                # and (on the second half) the row-tile DMA.  One psA tile
                # per half so consecutive halves pipeline on the 2-deep
                # 'mm' ring.
                ps = psA.tile([128, 512], F32, tag="mm", name=f"pj{it}_{ec}")
                for dt in range(DG // 128):
                    nc.tensor.matmul(
                        ps[:],
                        aoT_s[:, dt, it * 128:(it + 1) * 128],
                        wp_t[:, dt, ec * 512:(ec + 1) * 512],
                        start=(dt == 0), stop=(dt == DG // 128 - 1))
                if ec == 0:
                    yt_cur[it] = yp.tile([128, C], BF16, tag="y",
                                         name=f"yt{it}")
                yt = yt_cur[it]
                nc.vector.tensor_copy(yt[:, ec * 512:(ec + 1) * 512], ps[:])
                if ec == 1:
                    yt_cur.pop(it)
                    yqueue(it).dma_start(y[it * 128:(it + 1) * 128, :], yt[:])

            def proj_full_S(it):
                # one full y row-tile accumulated in a psS-ring tile (free
                # after the last se) -- interleaving these with psA halves
                # gives the tail proj 4-deep PSUM pipelining
                ps = psS.tile([128, 1024], F32, tag="st", name=f"pjS{it}")
                for ec in range(2):
                    for dt in range(DG // 128):
                        nc.tensor.matmul(
                            ps[:, ec * 512:(ec + 1) * 512],
                            aoT_s[:, dt, it * 128:(it + 1) * 128],
                            wp_t[:, dt, ec * 512:(ec + 1) * 512],
                            start=(dt == 0), stop=(dt == DG // 128 - 1))
                yt = yp.tile([128, C], BF16, tag="y", name=f"ytS{it}")
                nc.vector.tensor_copy(yt[:, 0:512], ps[:, 0:512])
                nc.vector.tensor_copy(yt[:, 512:1024], ps[:, 512:1024])
                yqueue(it).dma_start(y[it * 128:(it + 1) * 128, :], yt[:])

            # ---- emission schedule: chunk-major, pair-interleaved -------
            # column order (0,0),(1,0),(0,1),(1,1),... ; se leads av by
            # TRAIL steps; q/k production and vhats are emitted just in
            # time; after both pairs of a chunk norm, its 4 proj tiles
            # drip out one per step.  The first 8 se blocks are permuted
            # so each block's x quarters (kT chunk b, qT chunk ic) have
            # landed by the time the PE reaches it -- the exp stream then
            # runs stall-free from ~13us.
            COLS = [(p, ic) for ic in range(IC) for p in range(2)]
            av_list = [(p, ic, b) for (p, ic) in COLS for b in range(4)]
            se_order = [(0, 0, 0), (0, 0, 1), (1, 0, 0), (0, 0, 2),
                        (1, 0, 1), (0, 0, 3), (1, 0, 2), (1, 0, 3)] + \
                       [(p, ic, b) for (p, ic) in COLS[2:] for b in range(4)]
            TRAIL = 3
            se_step = {}

            k_done, q_done, vh_done = set(), set(), set()
            normed = set()
            proj_pending = []

            WK = {0: wk0_t, 1: wk1_t}
            WQ = {0: wq0_t, 1: wq1_t}

            def need_se(p, ic, b):
                if (p, b) not in k_done:
                    k_done.add((p, b))
                    qk_chunk(WK[p], kT_s, p, b)
                if (p, ic) not in q_done:
                    q_done.add((p, ic))
                    qk_chunk(WQ[p], qT_s, p, ic, bias=True)

            def need_vh(blk):
                if blk not in vh_done:
                    vh_done.add(blk)
                    for jt in range(4 * blk, 4 * blk + 4):
                        vhat(jt)

            def do_av(idx, half):
                p, ic, b = av_list[idx]
                need_vh(b)
                av(p, ic, 4 * b + 2 * half, 4 * b + 2 * half + 2)
                if b == 3 and half == 1:
                    norm(p, ic)
                    normed.add((p, ic))
                    if (1 - p, ic) in normed:
                        proj_pending.extend(
                            (it, ec) for it in range(4 * ic, 4 * ic + 4)
                            for ec in range(2))

            L = len(av_list)
            ai = 0
            need_se(*se_order[0])
            for i, sblk in enumerate(se_order):
                p, ic, b = sblk
                for jc in range(4 * b, 4 * b + 4):
                    se_pair(p, ic, jc)
                se_step[sblk] = i

                def av_ready():
                    return (ai < L - 2 and av_list[ai] in se_step
                            and se_step[av_list[ai]] <= i - TRAIL)

                pops = 0
                prefetched = False
                while av_ready() and pops < 2:
                    do_av(ai, 0)
                    # production for LATER steps sits between the av
                    # halves so ACT has stream to chew meanwhile
                    if not prefetched and i + 1 < len(se_order):
                        need_se(*se_order[i + 1])
                        prefetched = True
                    do_av(ai, 1)
                    ai += 1
                    pops += 1
                if not prefetched and i + 1 < len(se_order):
                    need_se(*se_order[i + 1])
                if ai < L:
                    need_vh(av_list[ai][2])
                if proj_pending:
                    proj_half(*proj_pending.pop(0))
            while ai < L - 2:
                do_av(ai, 0)
                do_av(ai, 1)
                ai += 1
                if proj_pending:
                    proj_half(*proj_pending.pop(0))
            # epilogue: the last column's remaining av blocks run per-head
            # so h0's norm chain (DVE/gpsimd) overlaps h1's AV matmuls
            for hs in ((0,), (1,)):
                for j in range(L - 2, L):
                    p_, ic_, b_ = av_list[j]
                    av(p_, ic_, 4 * b_, 4 * b_ + 4, hs=hs)
                norm(p_, ic_, hs=hs)
            normed.add((p_, ic_))
            # drain any leftover earlier-column halves first
            while proj_pending:
                proj_half(*proj_pending.pop(0))
            # final column: alternate psS full-tiles with psA half-pairs
            # so the drain pipelines 4 deep across both PSUM rings
            for k, it in enumerate(range(4 * ic_, 4 * ic_ + 4)):
                if k % 2 == 0:
                    proj_full_S(it)
                else:
                    proj_half(it, 0)
                    proj_half(it, 1)

    nc.compile()
    return nc


def _get_nc():
    if "nc" not in _CACHE:
        _CACHE["nc"] = _build()
    return _CACHE["nc"]


def kernel(x, qkv_w, qkv_b, proj_w, proj_b):
    global LAST_RESULTS
    x = np.asarray(x, dtype=np.float32)
    qkv_w = np.asarray(qkv_w, dtype=np.float32)
    qkv_b = np.asarray(qkv_b, dtype=np.float32)
    proj_w = np.asarray(proj_w, dtype=np.float32)
    proj_b = np.asarray(proj_b, dtype=np.float32)

    nc = _get_nc()
    bf16 = ml_dtypes.bfloat16

    wqT_f = qkv_w[0:C].T                # [C, C]
    wkT_f = qkv_w[C:2 * C].T
    wvT_f = qkv_w[2 * C:3 * C].T
    wpT_f = proj_w.T                    # [C, C]

    def tile128(a):
        # [C, W] -> [128, CT, W] with partition = c % 128, ct = c // 128
        w = a.shape[1]
        return np.ascontiguousarray(
            a.reshape(CT, 128, w).transpose(1, 0, 2))

    in_maps = []
    for c in range(NCORES):
        b, g = divmod(c, HG)
        ds = g * DG
        wq_g = tile128(wqT_f[:, ds:ds + DG]).astype(bf16)  # [128, CT, 256]
        wk_g = tile128(wkT_f[:, ds:ds + DG]).astype(bf16)
        wp_g = np.ascontiguousarray(
            wpT_f[ds:ds + DG].reshape(2, 128, C).transpose(1, 0, 2)).astype(bf16)
        # qbT: per-partition q bias, column dt = head pair
        qbT = np.ascontiguousarray(
            qkv_b[ds:ds + DG].reshape(2, 128).T, dtype=np.float32)
        # xT quarter-major: [128, 4, CT, 512]; partition = c % 128
        xq = x[b].T.reshape(CT, 128, 4, 512).transpose(1, 2, 0, 3)
        in_maps.append({
            "xT": np.ascontiguousarray(xq).astype(bf16),
            "wq0": np.ascontiguousarray(wq_g[:, :, 0:128]),
            "wq1": np.ascontiguousarray(wq_g[:, :, 128:256]),
            "wk0": np.ascontiguousarray(wk_g[:, :, 0:128]),
            "wk1": np.ascontiguousarray(wk_g[:, :, 128:256]),
            "wv": tile128(wvT_f[:, ds:ds + DG]).astype(bf16),
            "wp": wp_g,
            "qbT": qbT,
        })

    LAST_RESULTS = run_bass_kernel_spmd(nc, in_maps, list(range(NCORES)))
    # host unshard: sum the 4 partial projections per batch (f32 accumulate
    # of bf16 partials) and add the folded bias (proj_b + v_bias @ proj_w.T
    # -- exact, since sum(attn)=1)
    out_bias = proj_b + qkv_b[2 * C:3 * C] @ proj_w.T
    out = np.empty((B, N, C), np.float32)
    for b in range(B):
        acc = LAST_RESULTS.results[b * HG]["y"].astype(np.float32)
        for g in range(1, HG):
            acc = acc + LAST_RESULTS.results[b * HG + g]["y"].astype(np.float32)
        out[b] = acc + out_bias
    return out


# revision 50
# speedup vs baseline: 1.0229x; 1.0083x over previous
"""Multi-head attention forward on 8 Trainium2 NeuronCores.

Reference computation (B=2, N=2048, C=1024, H=16, Dh=64):
    qkv = x @ qkv_w.T + qkv_b            -> q, k, v per head
    attn = softmax(q @ k.T / sqrt(Dh))
    out  = (attn @ v) reassembled, then out @ proj_w.T + proj_b

Sharding: 8 cores = 2 batches x 4 head groups (tensor parallel on heads,
data parallel on batch).  Each core computes q/k/v for its 4 heads over
its batch's 2048 tokens, attention for those heads, and a partial
projection with its head-group's rows of proj_w.  The host sums the 4
partial projections per batch (bf16 partials, f32 accumulate) and adds
the (host-folded) proj + v biases.

Schedule: everything runs in the S^T orientation (S^T[j,i] = sum_d
kT[d,j] qT[d,i]) so softmax reductions over keys happen via matmul -- a
ones column in v-hat yields the denominator as row 64 of the AV PSUM
tile.  The kernel sits on the PE/ACT ridge (~155us PE floor, ~147us ACT
floor), so the schedule is chunk-major and PAIR-INTERLEAVED: head pairs
alternate per query chunk so both pairs of a chunk finish close together
and the projection + y DMA spread through the whole kernel instead of
piling up in a drain tail.  exp'd score tiles (es) buffer in SBUF,
decoupling the S+exp stream from the AV stream.  All matmul operands
are bf16.  Normalization evicts the ones-row + raw ao from PSUM
immediately (freeing the AV accumulators), takes the reciprocal on DVE,
broadcasts it across partitions on gpsimd, and rescales into aoT -- no
PE broadcast matmuls.  y is written bf16 (host sums partials in f32),
halving the output DMA.  Softmax max-subtraction is skipped
(S ~ N(0,1)).  The k bias is dropped (softmax-invariant); v/proj biases
are folded on the host.
"""

import sys

if "/opt/trn_rl_repo" not in sys.path:
    sys.path.insert(0, "/opt/trn_rl_repo")

from contextlib import ExitStack

import ml_dtypes
import numpy as np

from concourse import bacc, mybir, tile
from concourse.bass_utils import run_bass_kernel_spmd

F32 = mybir.dt.float32
BF16 = mybir.dt.bfloat16
AF = mybir.ActivationFunctionType

B, N, C, H, DH = 2, 2048, 1024, 16, 64
NCORES = 8
HG = 4              # head groups (cores per batch)
HPG = H // HG       # 4 heads per core
DG = HPG * DH       # 256 projected dims per core
CT = C // 128       # 8 contraction tiles
JT = N // 128       # 16 key tiles
IC = N // 512       # 4 query chunks
SCALE = DH ** -0.5

_CACHE = {}
LAST_RESULTS = None


def _build():
    nc = bacc.Bacc("TRN2", target_bir_lowering=False, debug=False,
                   num_devices=NCORES)

    # x quarter-major on BOTH sides: [128, quarter, ct, 512] so each
    # 512-token quarter is one DMA with contiguous 8KB per-partition lines
    xT = nc.dram_tensor("xT", [128, 4, CT, 512], BF16, kind="ExternalInput").ap()
    wq0 = nc.dram_tensor("wq0", [128, CT, 128], BF16, kind="ExternalInput").ap()
    wq1 = nc.dram_tensor("wq1", [128, CT, 128], BF16, kind="ExternalInput").ap()
    wk0 = nc.dram_tensor("wk0", [128, CT, 128], BF16, kind="ExternalInput").ap()
    wk1 = nc.dram_tensor("wk1", [128, CT, 128], BF16, kind="ExternalInput").ap()
    wv = nc.dram_tensor("wv", [128, CT, DG], BF16, kind="ExternalInput").ap()
    wp = nc.dram_tensor("wp", [128, DG // 128, C], BF16, kind="ExternalInput").ap()
    qbT = nc.dram_tensor("qbT", [128, 2], F32, kind="ExternalInput").ap()
    y = nc.dram_tensor("y", [N, C], BF16, kind="ExternalOutput").ap()

    with tile.TileContext(nc) as tc, ExitStack() as ctx:
        per = ctx.enter_context(tc.tile_pool(name="per", bufs=1))
        xT_s = per.tile([128, 4, CT, 512], BF16, tag="xT")
        qT_s = per.tile([128, 2, N], BF16, tag="qT")
        kT_s = per.tile([128, 2, N], BF16, tag="kT")
        vh_s = per.tile([128, JT, HPG, DH + 1], BF16, tag="vh")
        aoT_s = per.tile([128, 2, N], BF16, tag="aoT")
        wq0_t = per.tile([128, CT, 128], BF16, tag="wq0")
        wq1_t = per.tile([128, CT, 128], BF16, tag="wq1")
        wk0_t = per.tile([128, CT, 128], BF16, tag="wk0")
        wk1_t = per.tile([128, CT, 128], BF16, tag="wk1")
        wv_t = per.tile([128, CT, DG], BF16, tag="wv")
        wp_t = per.tile([128, DG // 128, C], BF16, tag="wp")
        qbT_s = per.tile([128, 2], F32, tag="qbT")
        ones_s = per.tile([1, 512], BF16, tag="ones")
        warm = per.tile([1, 16], F32, tag="warm")

        # ---- DMA emission.  Queues: sync + gpsimd (cheap trigger) +
        # vector carry inputs; the scalar (ACT) queue stays free for exp.
        def xdma(q, quarter, ct0, ct1):
            q.dma_start(xT_s[:, quarter, ct0:ct1, :],
                        xT[:, quarter, ct0:ct1, :])

        # Per-queue bandwidth is ~130-160 GB/s, so the critical first-chunk
        # set (wk0 + wq0 + x quarter 0) is split across all three DMA-capable
        # queues, and the x quarters stream in consumption order.  ones is
        # memset on-device (no DMA) so the warm matmuls start immediately.
        nc.gpsimd.memset(ones_s[:], 1.0)
        nc.sync.dma_start(wk0_t[:], wk0)
        nc.gpsimd.dma_start(wq0_t[:], wq0)
        xdma(nc.scalar, 0, 0, 3)
        xdma(nc.sync, 0, 3, 6)
        xdma(nc.gpsimd, 0, 6, 8)
        nc.gpsimd.dma_start(qbT_s[:], qbT)
        nc.scalar.dma_start(wk1_t[:], wk1)
        xdma(nc.scalar, 1, 0, 4)
        xdma(nc.gpsimd, 1, 4, 8)
        nc.gpsimd.dma_start(wq1_t[:], wq1)
        xdma(nc.scalar, 2, 0, 4)
        xdma(nc.sync, 2, 4, 8)
        nc.sync.dma_start(wv_t[:], wv)
        xdma(nc.gpsimd, 3, 0, 4)
        xdma(nc.scalar, 3, 4, 8)
        nc.gpsimd.dma_start(wp_t[:], wp)

        with tc.tile_pool(name="es", bufs=24) as esp, \
             tc.tile_pool(name="sm", bufs=4) as sm2, \
             tc.tile_pool(name="yp", bufs=3) as yp, \
             tc.tile_pool(name="psA", bufs=2, space="PSUM") as psA, \
             tc.tile_pool(name="psS", bufs=2, space="PSUM") as psS, \
             tc.tile_pool(name="psB", bufs=2, space="PSUM") as psB:

            # warm the exp table while the bulk DMAs run, and spin the
            # PE clock up to full rate (dense dummy matmuls overlap the
            # first x-chunk DMAs, so production starts warm)
            nc.scalar.activation(warm[:], ones_s[:, 0:16], AF.Exp)
            jk0 = psA.tile([128, 512], F32, tag="mm", name="jk0")
            for _ in range(6):
                nc.tensor.matmul(jk0[:], ones_s[:, 0:128], ones_s[:],
                                 start=True, stop=True)
            nc.gpsimd.memset(vh_s[:, :, :, DH], 1.0)

            def qk_chunk(w_t, dst, dt, nck, bias=False):
                ps = psA.tile([128, 512], F32, tag="mm")
                for ct in range(CT):
                    nc.tensor.matmul(
                        ps[:], w_t[:, ct, :],
                        xT_s[:, nck, ct, :],
                        start=(ct == 0), stop=(ct == CT - 1))
                out = dst[:, dt, nck * 512:(nck + 1) * 512]
                if bias:
                    nc.vector.tensor_scalar_add(out, ps[:],
                                                qbT_s[:, dt:dt + 1])
                else:
                    nc.vector.tensor_copy(out, ps[:])

            def vhat(jt):
                ps = psA.tile([128, 512], F32, tag="mm")
                q, r = divmod(jt, 4)
                for ct in range(CT):
                    nc.tensor.matmul(ps[:, 0:DG],
                                     xT_s[:, q, ct, r * 128:(r + 1) * 128],
                                     wv_t[:, ct, :],
                                     start=(ct == 0), stop=(ct == CT - 1))
                for h in range(HPG):
                    nc.vector.tensor_copy(vh_s[:, jt, h, 0:DH],
                                          ps[:, h * DH:(h + 1) * DH])

            es_store = {}
            av_tiles = {}

            def se_pair(p, ic, jc):
                # S^T row-group pair + exp for one key tile of chunk ic
                i0 = ic * 512
                st = psS.tile([128, 1024], F32, tag="st")
                nc.tensor.matmul(st[:, 0:512],
                                 kT_s[0:64, p, jc * 128:(jc + 1) * 128],
                                 qT_s[0:64, p, i0:i0 + 512],
                                 start=True, stop=True)
                nc.tensor.matmul(st[:, 512:1024],
                                 kT_s[64:128, p, jc * 128:(jc + 1) * 128],
                                 qT_s[64:128, p, i0:i0 + 512],
                                 start=True, stop=True)
                es = esp.tile([128, 1024], BF16, tag="es",
                              name=f"es{p}_{ic}_{jc}")
                es_store[(p, ic, jc)] = es
                nc.scalar.activation(es[:], st[:], AF.Exp, scale=SCALE)

            def av(p, ic, jc0, jc1, hs=(0, 1)):
                if (p, ic) not in av_tiles:
                    av_tiles[(p, ic)] = [
                        psB.tile([DH + 1, 512], F32, tag="outT",
                                 name=f"o{p}_{ic}{s}") for s in "ab"]
                outs = av_tiles[(p, ic)]
                for jc in range(jc0, jc1):
                    es = (es_store.pop((p, ic, jc)) if 1 in hs
                          else es_store[(p, ic, jc)])
                    for h in hs:
                        nc.tensor.matmul(
                            outs[h][:], vh_s[:, jc, 2 * p + h, :],
                            es[:, h * 512:(h + 1) * 512],
                            start=(jc == 0), stop=(jc == JT - 1))

            def norm(p, ic, hs=(0, 1), act_evict=False):
                # evict den + raw ao immediately (frees the psB tiles for
                # the next column's AV), then reciprocal on DVE, broadcast
                # on gpsimd, and normalize into aoT.  No PE work.  In the
                # epilogue the evictions run on the by-then-idle ACT
                # engine so the per-head chains parallelize.
                i0 = ic * 512
                outs = av_tiles[(p, ic)]
                if 1 in hs:
                    av_tiles.pop((p, ic))
                raws, recs = [], []
                for hi in hs:
                    outT = outs[hi]
                    den = sm2.tile([1, 512], F32, tag="den")
                    raw = sm2.tile([64, 512], F32, tag="raw")
                    if act_evict:
                        nc.scalar.copy(den[:], outT[64:65, :])
                        nc.scalar.copy(raw[:], outT[0:64, :])
                    else:
                        nc.vector.tensor_copy(den[:], outT[64:65, :])
                        nc.vector.tensor_copy(raw[:], outT[0:64, :])
                    rec = sm2.tile([1, 512], F32, tag="rec")
                    nc.vector.reciprocal_approx_fast(rec[:], den[:])
                    raws.append(raw)
                    recs.append(rec)
                bcs = []
                for k in range(len(hs)):
                    bc = sm2.tile([64, 512], F32, tag="bc")
                    nc.gpsimd.partition_broadcast(bc[:], recs[k][:])
                    bcs.append(bc)
                for k, hi in enumerate(hs):
                    ao = aoT_s[hi * 64:hi * 64 + 64, p, i0:i0 + 512]
                    nc.vector.tensor_mul(ao, raws[k][:], bcs[k][:])

            # y DMA queues: sync/gpsimd alternate; the last column's tiles
            # fan out over three queues (ACT is idle by then)
            def yqueue(it):
                if it < 12:
                    return nc.sync if it % 2 == 0 else nc.gpsimd
                return {12: nc.sync, 13: nc.gpsimd,
                        14: nc.scalar, 15: nc.sync}[it]

            yt_cur = {}

            def proj_half(it, ec):
                # half a 128-row tile of y: 2 accumulating matmuls, cast,
                # and (on the second half) the row-tile DMA.  One psA tile
                # per half so consecutive halves pipeline on the 2-deep
                # 'mm' ring.
                ps = psA.tile([128, 512], F32, tag="mm", name=f"pj{it}_{ec}")
                for dt in range(DG // 128):
                    nc.tensor.matmul(
                        ps[:],
                        aoT_s[:, dt, it * 128:(it + 1) * 128],
                        wp_t[:, dt, ec * 512:(ec + 1) * 512],
                        start=(dt == 0), stop=(dt == DG // 128 - 1))
                if ec == 0:
                    yt_cur[it] = yp.tile([128, C], BF16, tag="y",
                                         name=f"yt{it}")
                yt = yt_cur[it]
                nc.vector.tensor_copy(yt[:, ec * 512:(ec + 1) * 512], ps[:])
                if ec == 1:
                    yt_cur.pop(it)
                    yqueue(it).dma_start(y[it * 128:(it + 1) * 128, :], yt[:])

            def proj_full_S(it):
                # one full y row-tile accumulated in a psS-ring tile (free
                # after the last se) -- interleaving these with psA halves
                # gives the tail proj 4-deep PSUM pipelining
                ps = psS.tile([128, 1024], F32, tag="st", name=f"pjS{it}")
                for ec in range(2):
                    for dt in range(DG // 128):
                        nc.tensor.matmul(
                            ps[:, ec * 512:(ec + 1) * 512],
                            aoT_s[:, dt, it * 128:(it + 1) * 128],
                            wp_t[:, dt, ec * 512:(ec + 1) * 512],
                            start=(dt == 0), stop=(dt == DG // 128 - 1))
                yt = yp.tile([128, C], BF16, tag="y", name=f"ytS{it}")
                nc.vector.tensor_copy(yt[:, 0:512], ps[:, 0:512])
                nc.vector.tensor_copy(yt[:, 512:1024], ps[:, 512:1024])
                yqueue(it).dma_start(y[it * 128:(it + 1) * 128, :], yt[:])

            # ---- emission schedule: chunk-major, pair-interleaved -------
            # column order (0,0),(1,0),(0,1),(1,1),... ; se leads av by
            # TRAIL steps; q/k production and vhats are emitted just in
            # time; after both pairs of a chunk norm, its 4 proj tiles
            # drip out one per step.  The first 8 se blocks are permuted
            # so each block's x quarters (kT chunk b, qT chunk ic) have
            # landed by the time the PE reaches it -- the exp stream then
            # runs stall-free from ~13us.
            COLS = [(p, ic) for ic in range(IC) for p in range(2)]
            av_list = [(p, ic, b) for (p, ic) in COLS for b in range(4)]
            se_order = [(0, 0, 0), (0, 0, 1), (1, 0, 0), (0, 0, 2),
                        (1, 0, 1), (0, 0, 3), (1, 0, 2), (1, 0, 3)] + \
                       [(p, ic, b) for (p, ic) in COLS[2:] for b in range(4)]
            TRAIL = 3
            se_step = {}

            k_done, q_done, vh_done = set(), set(), set()
            normed = set()
            proj_pending = []

            WK = {0: wk0_t, 1: wk1_t}
            WQ = {0: wq0_t, 1: wq1_t}

            def need_se(p, ic, b):
                if (p, b) not in k_done:
                    k_done.add((p, b))
                    qk_chunk(WK[p], kT_s, p, b)
                if (p, ic) not in q_done:
                    q_done.add((p, ic))
                    qk_chunk(WQ[p], qT_s, p, ic, bias=True)

            def need_vh(blk):
                if blk not in vh_done:
                    vh_done.add(blk)
                    for jt in range(4 * blk, 4 * blk + 4):
                        vhat(jt)

            def do_av(idx, half):
                p, ic, b = av_list[idx]
                need_vh(b)
                av(p, ic, 4 * b + 2 * half, 4 * b + 2 * half + 2)
                if b == 3 and half == 1:
                    norm(p, ic)
                    normed.add((p, ic))
                    if (1 - p, ic) in normed:
                        proj_pending.extend(
                            (it, ec) for it in range(4 * ic, 4 * ic + 4)
                            for ec in range(2))

            L = len(av_list)
            ai = 0
            need_se(*se_order[0])
            for i, sblk in enumerate(se_order):
                p, ic, b = sblk
                for jc in range(4 * b, 4 * b + 4):
                    se_pair(p, ic, jc)
                se_step[sblk] = i

                def av_ready():
                    return (ai < L - 2 and av_list[ai] in se_step
                            and se_step[av_list[ai]] <= i - TRAIL)

                pops = 0
                prefetched = False
                while av_ready() and pops < 2:
                    do_av(ai, 0)
                    # production for LATER steps sits between the av
                    # halves so ACT has stream to chew meanwhile
                    if not prefetched and i + 1 < len(se_order):
                        need_se(*se_order[i + 1])
                        prefetched = True
                    do_av(ai, 1)
                    ai += 1
                    pops += 1
                if not prefetched and i + 1 < len(se_order):
                    need_se(*se_order[i + 1])
                if ai < L:
                    need_vh(av_list[ai][2])
                if proj_pending:
                    proj_half(*proj_pending.pop(0))
            while ai < L - 2:
                do_av(ai, 0)
                do_av(ai, 1)
                ai += 1
                if proj_pending:
                    proj_half(*proj_pending.pop(0))
            # epilogue: the last column's remaining av blocks run per-head
            # so h0's norm chain (DVE/gpsimd) overlaps h1's AV matmuls
            for hs in ((0,), (1,)):
                for j in range(L - 2, L):
                    p_, ic_, b_ = av_list[j]
                    av(p_, ic_, 4 * b_, 4 * b_ + 4, hs=hs)
                norm(p_, ic_, hs=hs, act_evict=True)
            normed.add((p_, ic_))
            # drain any leftover earlier-column halves first
            while proj_pending:
                proj_half(*proj_pending.pop(0))
            # final column: alternate psS full-tiles with psA half-pairs
            # so the drain pipelines 4 deep across both PSUM rings
            for k, it in enumerate(range(4 * ic_, 4 * ic_ + 4)):
                if k % 2 == 0:
                    proj_full_S(it)
                else:
                    proj_half(it, 0)
                    proj_half(it, 1)

    nc.compile()
    return nc


def _get_nc():
    if "nc" not in _CACHE:
        _CACHE["nc"] = _build()
    return _CACHE["nc"]


def kernel(x, qkv_w, qkv_b, proj_w, proj_b):
    global LAST_RESULTS
    x = np.asarray(x, dtype=np.float32)
    qkv_w = np.asarray(qkv_w, dtype=np.float32)
    qkv_b = np.asarray(qkv_b, dtype=np.float32)
    proj_w = np.asarray(proj_w, dtype=np.float32)
    proj_b = np.asarray(proj_b, dtype=np.float32)

    nc = _get_nc()
    bf16 = ml_dtypes.bfloat16

    wqT_f = qkv_w[0:C].T                # [C, C]
    wkT_f = qkv_w[C:2 * C].T
    wvT_f = qkv_w[2 * C:3 * C].T
    wpT_f = proj_w.T                    # [C, C]

    def tile128(a):
        # [C, W] -> [128, CT, W] with partition = c % 128, ct = c // 128
        w = a.shape[1]
        return np.ascontiguousarray(
            a.reshape(CT, 128, w).transpose(1, 0, 2))

    in_maps = []
    for c in range(NCORES):
        b, g = divmod(c, HG)
        ds = g * DG
        wq_g = tile128(wqT_f[:, ds:ds + DG]).astype(bf16)  # [128, CT, 256]
        wk_g = tile128(wkT_f[:, ds:ds + DG]).astype(bf16)
        wp_g = np.ascontiguousarray(
            wpT_f[ds:ds + DG].reshape(2, 128, C).transpose(1, 0, 2)).astype(bf16)
        # qbT: per-partition q bias, column dt = head pair
        qbT = np.ascontiguousarray(
            qkv_b[ds:ds + DG].reshape(2, 128).T, dtype=np.float32)
        # xT quarter-major: [128, 4, CT, 512]; partition = c % 128
        xq = x[b].T.reshape(CT, 128, 4, 512).transpose(1, 2, 0, 3)
        in_maps.append({
            "xT": np.ascontiguousarray(xq).astype(bf16),
            "wq0": np.ascontiguousarray(wq_g[:, :, 0:128]),
            "wq1": np.ascontiguousarray(wq_g[:, :, 128:256]),
            "wk0": np.ascontiguousarray(wk_g[:, :, 0:128]),
            "wk1": np.ascontiguousarray(wk_g[:, :, 128:256]),
            "wv": tile128(wvT_f[:, ds:ds + DG]).astype(bf16),
            "wp": wp_g,
            "qbT": qbT,
        })

    LAST_RESULTS = run_bass_kernel_spmd(nc, in_maps, list(range(NCORES)))
    # host unshard: sum the 4 partial projections per batch (f32 accumulate
    # of bf16 partials) and add the folded bias (proj_b + v_bias @ proj_w.T
    # -- exact, since sum(attn)=1)
    out_bias = proj_b + qkv_b[2 * C:3 * C] @ proj_w.T
    out = np.empty((B, N, C), np.float32)
    for b in range(B):
        acc = LAST_RESULTS.results[b * HG]["y"].astype(np.float32)
        for g in range(1, HG):
            acc = acc + LAST_RESULTS.results[b * HG + g]["y"].astype(np.float32)
        out[b] = acc + out_bias
    return out
